# revision 6
# baseline (speedup 1.0000x reference)
"""SpGAT_Conv Trainium2 kernel: 8-core SPMD spectral GNN conv.

Math (reference):
    a = softmax(alpha)
    pre = x @ W                                   [N, D]
    out_low  = s0 @ (a0 * (s1 @ pre))             [N, D]
    out_high = s2 @ (a1 * (s3 @ pre))             [N, D]
    out = relu(max(out_low, out_high) + bias)

Sharding: row-shard the node dim N across 8 cores.  Let S = concat(s1, s3)
(rows 0..N-1).  Core c owns rows [1024c, 1024c+1024):
    phase 1: pre_c = x_c @ W          -> AllGather pre
    phase 2: t_c   = S_c @ pre        -> AllGather t   (t = concat(t1, t3))
    phase 3: out_c = relu(max(a0 * s0_c @ t1, a1 * s2_c @ t3) + bias)

All big operands are pre-transposed host-side during sharding so the PE's
contraction dim lands on SBUF partitions with plain contiguous DMAs
(fp32 has no DMA-transpose path on trn2).  Compute dtype is bf16
(host-cast; full PE rate) with fp32 PSUM accumulation; set
SPGAT_COMPUTE=f32r for the float32r variant.
"""

import os

import numpy as np

N_CORES = 8
N = 8192
K = 2048
NK = N - K          # 6144
D = 512
ROWS = N // N_CORES  # 1024 rows per core
P = 128
RCH = ROWS // P      # 8  (row chunks per core / output strips)
NCH = N // P         # 64 (contraction chunks over full N)
KCH = K // P         # 16 (low-band chunks; high band = NCH - KCH = 48)

COMPUTE = os.environ.get("SPGAT_COMPUTE", "bf16")  # "bf16" | "f32r"
DEBUG = os.environ.get("SPGAT_DEBUG", "0") == "1"

_CACHE = {}


def _build_nc(compute):
    import concourse.mybir as mybir
    import concourse.tile as tile
    from concourse import bacc

    f32 = mybir.dt.float32
    bf16 = mybir.dt.bfloat16
    f32r = mybir.dt.float32r
    cdt = bf16 if compute == "bf16" else f32   # storage dtype of matmul operands

    def mmcast(ap):
        # dtype the PE sees; f32 storage is reinterpreted as float32r (full rate)
        return ap.bitcast(f32r) if compute == "f32r" else ap

    nc = bacc.Bacc(
        "TRN2", target_bir_lowering=False, debug=False, num_devices=N_CORES
    )

    xt = nc.dram_tensor("xt", [D, ROWS], cdt, kind="ExternalInput").ap()
    w = nc.dram_tensor("w", [D, D], cdt, kind="ExternalInput").ap()
    alpha = nc.dram_tensor("alpha", [2], f32, kind="ExternalInput").ap()
    bias = nc.dram_tensor("bias", [D], f32, kind="ExternalInput").ap()
    st = nc.dram_tensor("st", [N, ROWS], cdt, kind="ExternalInput").ap()
    s0t = nc.dram_tensor("s0t", [K, ROWS], cdt, kind="ExternalInput").ap()
    s2t = nc.dram_tensor("s2t", [NK, ROWS], cdt, kind="ExternalInput").ap()
    out = nc.dram_tensor("out", [ROWS, D], f32, kind="ExternalOutput").ap()
    if DEBUG:
        pre_dump = nc.dram_tensor("pre_dump", [N, D], cdt, kind="ExternalOutput").ap()
        t_dump = nc.dram_tensor("t_dump", [N, D], cdt, kind="ExternalOutput").ap()
        a_dump = nc.dram_tensor("a_dump", [P, 2], f32, kind="ExternalOutput").ap()

    groups = [list(range(N_CORES))]

    with tile.TileContext(nc) as tc:
        with (
            tc.tile_pool(name="const", bufs=1) as const,
            tc.tile_pool(name="big", bufs=1) as big,
            tc.tile_pool(name="strips", bufs=4) as strips,
            tc.tile_pool(name="stage", bufs=4) as stage,
            tc.tile_pool(name="tmp", bufs=RCH) as tmpp,
            tc.tile_pool(name="ps", bufs=8, space="PSUM") as ps,
            tc.tile_pool(name="dram", bufs=1, space="DRAM") as dram,
        ):
            # ---- setup: softmax(alpha), broadcast a and bias to 128 partitions
            asb = const.tile([1, 2], f32, name="asb")
            nc.sync.dma_start(asb[:], alpha[None, :])
            bsb = const.tile([1, D], f32, name="bsb")
            nc.sync.dma_start(bsb[:], bias[None, :])

            amax = const.tile([1, 1], f32, name="amax")
            nc.vector.tensor_tensor(
                amax[:], asb[:, 0:1], asb[:, 1:2], mybir.AluOpType.max
            )
            ash = const.tile([1, 2], f32, name="ash")
            nc.vector.tensor_scalar(
                ash[:], asb[:], amax[:, 0:1], None, mybir.AluOpType.subtract
            )
            aexp = const.tile([1, 2], f32, name="aexp")
            nc.scalar.activation(aexp[:], ash[:], mybir.ActivationFunctionType.Exp)
            asum = const.tile([1, 1], f32, name="asum")
            nc.vector.tensor_tensor(
                asum[:], aexp[:, 0:1], aexp[:, 1:2], mybir.AluOpType.add
            )
            arec = const.tile([1, 1], f32, name="arec")
            nc.vector.reciprocal(arec[:], asum[:])
            afin = const.tile([1, 2], f32, name="afin")
            nc.vector.tensor_scalar(
                afin[:], aexp[:], arec[:, 0:1], None, mybir.AluOpType.mult
            )

            ones = const.tile([1, P], f32, name="ones")
            nc.vector.memset(ones[:], 1.0)
            ps_a = ps.tile([P, 2], f32, name="ps_a", tag="acc")
            nc.tensor.matmul(ps_a[:], ones[:], afin[:], start=True, stop=True)
            a128 = const.tile([P, 2], f32, name="a128")
            nc.vector.tensor_copy(a128[:], ps_a[:])
            ps_b = ps.tile([P, D], f32, name="ps_b", tag="acc")
            nc.tensor.matmul(ps_b[:], ones[:], bsb[:], start=True, stop=True)
            bias128 = const.tile([P, D], f32, name="bias128")
            nc.vector.tensor_copy(bias128[:], ps_b[:])

            # ---- phase 1: pre_c = x_c @ W
            xt_sb = const.tile([P, D // P, ROWS], cdt, name="xt_sb")
            nc.sync.dma_start(
                xt_sb[:], xt.rearrange("(c p) r -> p c r", p=P)
            )
            w_sb = const.tile([P, D // P, D], cdt, name="w_sb")
            nc.sync.dma_start(w_sb[:], w.rearrange("(c p) d -> p c d", p=P))

            pre_in = dram.tile([ROWS, D], cdt, name="pre_in")
            pre_out = dram.tile([N, D], cdt, name="pre_out", addr_space="Shared")

            for nb in range(RCH):
                acc = ps.tile([P, D], f32, name=f"acc1_{nb}", tag="acc")
                for dc in range(D // P):
                    nc.tensor.matmul(
                        acc[:],
                        mmcast(xt_sb[:, dc, P * nb : P * (nb + 1)]),
                        mmcast(w_sb[:, dc, :]),
                        start=(dc == 0),
                        stop=(dc == D // P - 1),
                    )
                pst = stage.tile([P, D], cdt, name=f"pre_st_{nb}", tag="st")
                nc.vector.tensor_copy(pst[:], acc[:])
                nc.sync.dma_start(pre_in[P * nb : P * (nb + 1), :], pst[:])

            nc.gpsimd.collective_compute(
                "AllGather",
                mybir.AluOpType.bypass,
                replica_groups=groups,
                ins=[pre_in.opt()],
                outs=[pre_out.opt()],
            )

            # ---- phase 2: t_c = S_c @ pre
            pre_sb = big.tile([P, NCH, D], cdt, name="pre_sb", tag="big")
            pre_v = pre_out.rearrange("(c p) d -> c p d", p=P)
            for j in range(NCH):
                nc.sync.dma_start(pre_sb[:, j, :], pre_v[j])

            t_in = dram.tile([ROWS, D], cdt, name="t_in")
            t_out = dram.tile([N, D], cdt, name="t_out", addr_space="Shared")

            accs2 = [
                ps.tile([P, D], f32, name=f"acc2_{kt}", tag="acc") for kt in range(RCH)
            ]
            for j in range(NCH):
                strip = strips.tile([P, ROWS], cdt, name=f"s_{j}", tag="strip")
                nc.sync.dma_start(strip[:], st[P * j : P * (j + 1), :])
                for kt in range(RCH):
                    nc.tensor.matmul(
                        accs2[kt][:],
                        mmcast(strip[:, P * kt : P * (kt + 1)]),
                        mmcast(pre_sb[:, j, :]),
                        start=(j == 0),
                        stop=(j == NCH - 1),
                    )
            for kt in range(RCH):
                tst = stage.tile([P, D], cdt, name=f"t_st_{kt}", tag="st")
                nc.vector.tensor_copy(tst[:], accs2[kt][:])
                nc.sync.dma_start(t_in[P * kt : P * (kt + 1), :], tst[:])

            nc.gpsimd.collective_compute(
                "AllGather",
                mybir.AluOpType.bypass,
                replica_groups=groups,
                ins=[t_in.opt()],
                outs=[t_out.opt()],
            )

            if DEBUG:
                nc.sync.dma_start(a_dump[:], a128[:])
                nc.sync.dma_start(pre_dump[:], pre_out[:])
                nc.sync.dma_start(t_dump[:], t_out[:])

            # ---- phase 3: out_c = relu(max(a0*s0_c@t1, a1*s2_c@t3) + bias)
            t_sb = big.tile([P, NCH, D], cdt, name="t_sb", tag="big")
            t_v = t_out.rearrange("(c p) d -> c p d", p=P)
            for j in range(NCH):
                nc.sync.dma_start(t_sb[:, j, :], t_v[j])

            accs3 = [
                ps.tile([P, D], f32, name=f"acc3_{nt}", tag="acc") for nt in range(RCH)
            ]
            tmps = [
                tmpp.tile([P, D], f32, name=f"low_{nt}", tag="tmp") for nt in range(RCH)
            ]
            # low band: s0_c @ t1   (contraction chunks 0..KCH-1)
            for j in range(NCH):
                strip = strips.tile([P, ROWS], cdt, name=f"r_{j}", tag="strip")
                if j < KCH:
                    nc.sync.dma_start(strip[:], s0t[P * j : P * (j + 1), :])
                else:
                    jj = j - KCH
                    nc.sync.dma_start(strip[:], s2t[P * jj : P * (jj + 1), :])
                for nt in range(RCH):
                    nc.tensor.matmul(
                        accs3[nt][:],
                        mmcast(strip[:, P * nt : P * (nt + 1)]),
                        mmcast(t_sb[:, j, :]),
                        start=(j == 0 or j == KCH),
                        stop=(j == KCH - 1 or j == NCH - 1),
                    )
                if j == KCH - 1:
                    # stash a0 * low so the PSUM banks can be reused for high
                    for nt in range(RCH):
                        nc.vector.tensor_scalar(
                            tmps[nt][:],
                            accs3[nt][:],
                            a128[:, 0:1],
                            None,
                            mybir.AluOpType.mult,
                        )

            for nt in range(RCH):
                hi = stage.tile([P, D], f32, name=f"hi_{nt}", tag="hi")
                nc.vector.tensor_scalar(
                    hi[:], accs3[nt][:], a128[:, 1:2], None, mybir.AluOpType.mult
                )
                nc.vector.tensor_tensor(hi[:], hi[:], tmps[nt][:], mybir.AluOpType.max)
                nc.vector.tensor_tensor(
                    hi[:], hi[:], bias128[:], mybir.AluOpType.add
                )
                osb = stage.tile([P, D], f32, name=f"osb_{nt}", tag="osb")
                nc.scalar.activation(
                    osb[:], hi[:], mybir.ActivationFunctionType.Relu
                )
                nc.sync.dma_start(out[P * nt : P * (nt + 1), :], osb[:])

    nc.compile()
    return nc


def _get_nc(compute):
    if compute not in _CACHE:
        _CACHE[compute] = _build_nc(compute)
    return _CACHE[compute]


def _shard_inputs(x, weights, alpha, bias, s0, s1, s2, s3, compute):
    import ml_dtypes

    cnp = ml_dtypes.bfloat16 if compute == "bf16" else np.float32

    def prep(a):  # transpose + cast, C-contiguous
        return np.ascontiguousarray(a.T).astype(cnp, copy=False)

    alpha = np.ascontiguousarray(alpha, dtype=np.float32)
    bias = np.ascontiguousarray(bias, dtype=np.float32)
    w_p = np.ascontiguousarray(weights).astype(cnp, copy=False)  # natural: rhs is contract-major
    in_maps = []
    for c in range(N_CORES):
        r0, r1 = ROWS * c, ROWS * (c + 1)
        # S = concat(s1, s3) rows; core c owns rows [r0, r1)
        if r1 <= K:
            s_rows = s1[r0:r1]
        elif r0 >= K:
            s_rows = s3[r0 - K : r1 - K]
        else:  # straddles the boundary (not the case for these shapes)
            s_rows = np.concatenate([s1[r0:], s3[: r1 - K]], axis=0)
        in_maps.append(
            {
                "xt": prep(x[r0:r1]),
                "w": w_p,
                "alpha": alpha,
                "bias": bias,
                "st": prep(s_rows),
                "s0t": prep(s0[r0:r1]),
                "s2t": prep(s2[r0:r1]),
            }
        )
    return in_maps


def kernel(x, weights, alpha, bias, s0, s1, s2, s3, _trace=False):
    from concourse.bass_utils import run_bass_kernel_spmd

    compute = COMPUTE
    nc = _get_nc(compute)
    in_maps = _shard_inputs(
        np.asarray(x), np.asarray(weights), np.asarray(alpha), np.asarray(bias),
        np.asarray(s0), np.asarray(s1), np.asarray(s2), np.asarray(s3), compute,
    )
    kwargs = {}
    if _trace:
        kwargs = dict(trace=True, trace_cores=list(range(N_CORES)))
    r = run_bass_kernel_spmd(nc, in_maps, core_ids=list(range(N_CORES)), **kwargs)
    full = np.concatenate([res["out"] for res in r.results], axis=0)
    if _trace:
        return full, r
    return full
